# revision 30
# baseline (speedup 1.0000x reference)
"""Causal GQA attention (B=2, T=2048, D=2048, QH=16, KVH=4, HD=128) on 8 TRN2 cores.

Sharding: DP-2 over batch x TP-4 over KV-head groups.
  core c -> batch c//4, kv head c%4, q heads 4*(c%4)..4*(c%4)+3.
Each core computes a partial (T, D) output (its heads' contribution through wo);
the host sums the 4 partials per batch (the all-reduce of the "wo along in dim"
sharding) and stacks the two batches.

Device dataflow (everything transposed; no on-device activation transposes):
  - host feeds xT = x[b].T                            (D, T)
  - qT/kT = W^T x computed directly in [hd, t] layout (wq chunks are lhsT)
  - RoPE via swap-permutation matmul (rot = R @ qT) + DVE mul/add with
    host cos / sign-folded-sin tables in [hd, t] layout
  - S^T[key, q] = (kT_blk)^T @ qT  per 128-key block  (one matmul, K=hd=128)
  - exp on ACT with fused 1/sqrt(hd) scale, PSUM -> SBUF f32r
  - causal: fully-masked column ranges of diagonal blocks are never computed;
    the 128x128 diagonal triangle is masked by a DVE multiply
  - O^T[hd, q] += V_blk^T @ expS^T   (V natural from 4 PE transposes per tile)
  - den_bcast[128, q] += ones128x128 @ expS^T  (accumulating matmul that sums
    over keys AND broadcasts the softmax denominator to all partitions)
  - normalize (deferred one head so the PE never stalls): den_bcast -> DVE
    evac -> chunked DVE reciprocal -> DVE multiply into O^T
  - out[t, d] = sum_h (OT_h)^T @ wo_h  accumulated over the 4 heads

All matmuls run in float16 (1 cycle/row on the PE; fp32 PSUM accumulation).
Measured on hardware: ~310 us HW exec, max rel err ~4.6e-4 vs the fp32
reference (scaled by output absmax).
"""
import numpy as np
from contextlib import ExitStack

import concourse.bacc as bacc
import concourse.tile as tile
import concourse.mybir as mybir
from concourse.bass_utils import run_bass_kernel_spmd

B, T, D = 2, 2048, 2048
QH, KVH = 16, 4
HD = D // QH            # 128
P = 128
NT = T // 512           # 4 t-tiles of 512
DC = D // P             # 16 contraction chunks
KB = T // P             # 16 key blocks
F32 = mybir.dt.float32
F32R = mybir.dt.float32r
CDT = mybir.dt.float16          # compute dtype on the PE (1 cycle/row)
NPDT = np.float16
AF = mybir.ActivationFunctionType
ALU = mybir.AluOpType
SCALE = float(1.0 / np.sqrt(HD))

_cached = {}


def _build():
    nc = bacc.Bacc("TRN2", target_bir_lowering=False, debug=False)
    xT = nc.dram_tensor("xT", [D, T], CDT, kind="ExternalInput")
    wq = nc.dram_tensor("wq", [D, 4 * HD], CDT, kind="ExternalInput")
    wk = nc.dram_tensor("wk", [D, HD], CDT, kind="ExternalInput")
    wv = nc.dram_tensor("wv", [D, HD], CDT, kind="ExternalInput")
    wo = nc.dram_tensor("wo", [4 * HD, D], CDT, kind="ExternalInput")
    cosT = nc.dram_tensor("cosT", [HD, T], CDT, kind="ExternalInput")
    ssinT = nc.dram_tensor("ssinT", [HD, T], CDT, kind="ExternalInput")
    rmat = nc.dram_tensor("rmat", [P, P], CDT, kind="ExternalInput")
    tri = nc.dram_tensor("tri", [P, P], CDT, kind="ExternalInput")
    ident = nc.dram_tensor("ident", [P, P], CDT, kind="ExternalInput")
    out = nc.dram_tensor("out", [T, D], F32, kind="ExternalOutput")

    with tile.TileContext(nc) as tc, ExitStack() as ctx:
        const = ctx.enter_context(tc.tile_pool(name="const", bufs=1))
        kvres = ctx.enter_context(tc.tile_pool(name="kvres", bufs=1))
        xc_pool = ctx.enter_context(tc.tile_pool(name="xc", bufs=6))
        qr_pool = ctx.enter_context(tc.tile_pool(name="qr", bufs=6))
        tmp_pool = ctx.enter_context(tc.tile_pool(name="tmp", bufs=3))
        e_pool = ctx.enter_context(tc.tile_pool(name="ep", bufs=8))
        ot_pool = ctx.enter_context(tc.tile_pool(name="ot", bufs=1))
        oev_pool = ctx.enter_context(tc.tile_pool(name="oev", bufs=4))
        bc_pool = ctx.enter_context(tc.tile_pool(name="bc", bufs=3))
        sm_pool = ctx.enter_context(tc.tile_pool(name="sm", bufs=2))

        ps_w = ctx.enter_context(tc.tile_pool(name="psw", bufs=5, space="PSUM"))
        ps_o = ctx.enter_context(tc.tile_pool(name="pso", bufs=2, space="PSUM"))
        ps_d = ctx.enter_context(tc.tile_pool(name="psd", bufs=1, space="PSUM"))

        # ---- resident constants (split + ordered for startup overlap) ----
        wq_sb = const.tile([P, DC, 4 * HD], CDT, tag="wq")
        wk_sb = const.tile([P, DC, HD], CDT, tag="wk")
        wv_sb = const.tile([P, DC, HD], CDT, tag="wv")
        wo_sb = const.tile([P, 4, D], CDT, tag="wo")
        cos_sb = const.tile([P, T], CDT, tag="cos")
        sin_sb = const.tile([P, T], CDT, tag="sin")
        rm_sb = const.tile([P, P], CDT, tag="rm")
        tri_sb = const.tile([P, P], CDT, tag="tri")
        id_sb = const.tile([P, P], CDT, tag="id")

        kT_all = kvres.tile([P, T], CDT, tag="kT")
        v_all = kvres.tile([P, KB, HD], CDT, tag="V")

        xT_v = xT.rearrange("(g dc p) t -> g p dc t", p=P, dc=4)   # 4-chunk groups
        wq_v = wq.rearrange("(g dc p) n -> g p dc n", p=P, dc=4)
        wk_v = wk.rearrange("(dc p) n -> p dc n", p=P)
        wv_v = wv.rearrange("(dc p) n -> p dc n", p=P)
        wo_v = wo.rearrange("(c p) n -> c p n", p=P)

        def load_x_tile(tt):
            grps = []
            for g in range(4):
                xg = xc_pool.tile([P, 4, 512], CDT, tag="xc", name=f"xc{tt}_{g}")
                nc.sync.dma_start(out=xg[:], in_=xT_v[g, :, :, tt * 512:(tt + 1) * 512])
                grps.append(xg)
            return [grps[dc // 4][:, dc % 4, :] for dc in range(DC)]

        # first t-tile's x groups interleaved with wq groups (startup critical);
        # the very first group is split in half so the first matmul starts early
        xgs0 = []
        for g in range(4):
            if g == 2:
                nc.sync.dma_start(out=cos_sb[:], in_=cosT[:])
                nc.sync.dma_start(out=sin_sb[:], in_=ssinT[:])
            xg = xc_pool.tile([P, 4, 512], CDT, tag="xc", name=f"xc0_{g}")
            if g == 0:
                nc.sync.dma_start(out=xg[:, 0:2, :], in_=xT_v[0, :, 0:2, 0:512])
                nc.sync.dma_start(out=wq_sb[:, 0:2, :], in_=wq_v[0, :, 0:2, :])
                nc.sync.dma_start(out=xg[:, 2:4, :], in_=xT_v[0, :, 2:4, 0:512])
                nc.sync.dma_start(out=wq_sb[:, 2:4, :], in_=wq_v[0, :, 2:4, :])
            else:
                nc.sync.dma_start(out=xg[:], in_=xT_v[g, :, :, 0:512])
                nc.sync.dma_start(out=wq_sb[:, g * 4:(g + 1) * 4, :], in_=wq_v[g])
            xgs0.append(xg)
        xcs0 = [xgs0[dc // 4][:, dc % 4, :] for dc in range(DC)]
        nc.sync.dma_start(out=wk_sb[:], in_=wk_v)
        nc.sync.dma_start(out=wv_sb[:], in_=wv_v)
        nc.sync.dma_start(out=rm_sb[:], in_=rmat[:])
        nc.sync.dma_start(out=tri_sb[:], in_=tri[:])
        nc.sync.dma_start(out=id_sb[:], in_=ident[:])
        ones_mat = const.tile([P, P], CDT, tag="ones")
        nc.vector.memset(ones_mat[:], 1.0)

        def rope(dst_ap, src_ps, tt, nm):
            """dst[hd, 512] = src*cos + (R@src)*ssin for t-tile tt. src is PSUM."""
            c_sl = cos_sb[:, tt * 512:(tt + 1) * 512]
            s_sl = sin_sb[:, tt * 512:(tt + 1) * 512]
            sb = tmp_pool.tile([P, 512], CDT, tag="evac", name=f"ev_{nm}")
            nc.scalar.copy(sb[:], src_ps[:])
            rot_ps = ps_w.tile([P, 512], F32, tag="w", name=f"rot_{nm}")
            nc.tensor.matmul(rot_ps[:], rm_sb[:], sb[:], start=True, stop=True)
            t1 = tmp_pool.tile([P, 512], F32, tag="t1", name=f"t1_{nm}")
            nc.vector.tensor_mul(t1[:], sb[:], c_sl)
            t2 = tmp_pool.tile([P, 512], F32, tag="t2", name=f"t2_{nm}")
            nc.vector.tensor_mul(t2[:], rot_ps[:], s_sl)
            with nc.allow_low_precision(reason="f32r rounding for PE"):
                nc.vector.tensor_add(dst_ap, t1[:], t2[:])

        def make_proj_thunks(tt, qT_roped):
            """A(tt) as a list of small emission thunks, paced into C(tt-1)."""
            xcs = xcs0 if tt == 0 else load_x_tile(tt)
            tsl = slice(tt * 512, (tt + 1) * 512)
            groups = [[("q", 0), ("q", 1)], [("q", 2), ("q", 3)],
                      [("k", 0), ("v", 0)]]
            thunks = []
            state = {}

            def finish_tgt(kind, idx, gi, tt=tt, tsl=tsl):
                ps = state[gi][(kind, idx)]
                if kind == "q":
                    rope(qT_roped[idx][:], ps, tt, f"q{tt}_{idx}")
                elif kind == "k":
                    rope(kT_all[:, tsl], ps, tt, f"k{tt}")
                else:
                    vt_sb = tmp_pool.tile([P, 512], CDT, tag="evac", name=f"vt{tt}")
                    nc.scalar.copy(vt_sb[:], ps[:])
                    tr_ps = ps_w.tile([P, 512], CDT, tag="w", name=f"vtr{tt}")
                    for i in range(4):
                        nc.tensor.transpose(tr_ps[:, i * P:(i + 1) * P],
                                            vt_sb[:, i * P:(i + 1) * P], id_sb[:])
                    for i in range(4):
                        with nc.allow_low_precision(reason="f32r store"):
                            nc.vector.tensor_copy(v_all[:, tt * 4 + i, :],
                                                  tr_ps[:, i * P:(i + 1) * P])

            for gi, grp in enumerate(groups):
                def alloc(grp=grp, gi=gi, tt=tt):
                    state[gi] = {
                        tgt: ps_w.tile([P, 512], F32, tag="w",
                                       name=f"proj{tt}_{tgt[0]}{tgt[1]}")
                        for tgt in grp}
                thunks.append(alloc)
                for dc in range(DC):
                    def mm(dc=dc, grp=grp, gi=gi):
                        for tgt in grp:
                            kind, idx = tgt
                            if kind == "q":
                                lhsT = wq_sb[:, dc, idx * HD:(idx + 1) * HD]
                            elif kind == "k":
                                lhsT = wk_sb[:, dc, :]
                            else:
                                lhsT = wv_sb[:, dc, :]
                            nc.tensor.matmul(state[gi][tgt][:], lhsT, xcs[dc],
                                             start=(dc == 0), stop=(dc == DC - 1))
                    thunks.append(mm)
                for tgt in grp:
                    thunks.append(lambda tgt=tgt, gi=gi: finish_tgt(*tgt, gi))
            return thunks

        qT_next = [qr_pool.tile([P, 512], CDT, tag="qr", name=f"qr0_{i}")
                   for i in range(4)]
        for th in make_proj_thunks(0, qT_next):   # A(0): nothing to hide behind
            th()
        thunks_next = []

        for tt in range(NT):
            tsl = slice(tt * 512, (tt + 1) * 512)
            qT_roped = qT_next
            if tt > 0:
                for th in make_proj_thunks(tt, qT_roped):
                    th()
            if tt + 1 < NT:
                qT_next = [qr_pool.tile([P, 512], CDT, tag="qr",
                                        name=f"qr{tt + 1}_{i}") for i in range(4)]

            # ---------- Phase B: attention, one-head-deferred normalization ----
            nkb = 4 * (tt + 1)
            ot_sb = ot_pool.tile([P, 4, 512], CDT, tag="ot", name=f"ot{tt}")
            pending_norm = []

            def finish_head(bc_ps, o_ps, hh, tt=tt, ot_sb=ot_sb):
                bc_sb = bc_pool.tile([P, 512], CDT, tag="bc", name=f"bs{tt}_{hh}")
                with nc.allow_low_precision(reason="bc evac"):
                    nc.vector.tensor_copy(bc_sb[:], bc_ps[:])
                rb_sb = bc_pool.tile([P, 512], CDT, tag="rb", name=f"rb{tt}_{hh}")
                # chunked so phase C's first consumers unblock after 1/4 of the
                # reciprocal instead of the whole 3.3us op
                for ch in range(4):
                    cs = slice(ch * P, (ch + 1) * P)
                    with nc.allow_low_precision(reason="recip"):
                        nc.vector.reciprocal(rb_sb[:, cs], bc_sb[:, cs])
                    with nc.allow_low_precision(reason="norm"):
                        nc.vector.tensor_mul(ot_sb[:, hh, cs], o_ps[:, cs],
                                             rb_sb[:, cs])

            for hh in range(4):
                o_ps = ps_o.tile([P, 512], F32, tag="o", name=f"o{tt}_{hh}")
                den_ps = ps_d.tile([P, 512], F32, tag="den", name=f"d{tt}_{hh}")
                prev = None   # (kb, lo, e_sb)
                for kb in range(nkb):
                    di = kb - 4 * tt          # >=0 on diagonal blocks
                    lo = di * P if di > 0 else 0
                    s_ps = ps_w.tile([P, 512], F32, tag="w", name=f"s{tt}_{hh}_{kb}")
                    nc.tensor.matmul(s_ps[:, lo:512],
                                     kT_all[:, kb * P:(kb + 1) * P],
                                     qT_roped[hh][:, lo:512], start=True, stop=True)
                    e_sb = e_pool.tile([P, 512], CDT, tag="e", name=f"e{tt}_{hh}_{kb}")
                    nc.scalar.activation(e_sb[:, lo:512], s_ps[:, lo:512], AF.Exp,
                                         scale=SCALE)
                    if di >= 0:
                        with nc.allow_low_precision(reason="mask mult"):
                            nc.vector.tensor_mul(e_sb[:, di * P:(di + 1) * P],
                                                 e_sb[:, di * P:(di + 1) * P],
                                                 tri_sb[:])
                    if prev is not None:
                        pkb, plo, pe = prev
                        nc.tensor.matmul(o_ps[:, plo:512], v_all[:, pkb, :],
                                         pe[:, plo:512],
                                         start=(pkb == 0), stop=False)
                        nc.tensor.matmul(den_ps[:, plo:512], ones_mat,
                                         pe[:, plo:512],
                                         start=(pkb == 0), stop=False)
                    prev = (kb, lo, e_sb)
                pkb, plo, pe = prev
                nc.tensor.matmul(o_ps[:, plo:512], v_all[:, pkb, :], pe[:, plo:512],
                                 start=(pkb == 0), stop=True)
                nc.tensor.matmul(den_ps[:, plo:512], ones_mat, pe[:, plo:512],
                                 start=(pkb == 0), stop=True)
                # den_ps already holds the broadcast denominators; defer
                # evac+recip+norm one head so the PE never waits
                if pending_norm:
                    finish_head(*pending_norm.pop())
                pending_norm.append((den_ps, o_ps, hh))
            last_norm = pending_norm.pop()

            if tt == 0:
                # wo arrives late on purpose: keeps startup DMA bandwidth for
                # the tensors the first projections need
                for c in range(4):
                    nc.sync.dma_start(out=wo_sb[:, c, :], in_=wo_v[c])

            # ---------- Phase C: output projection ----------
            # First two f-groups: emit heads 0-2, then the deferred head-3
            # normalization (its reciprocal overlaps these matmuls), then the
            # head-3 contributions.
            lead = []
            for tc4 in range(4):
                trow = tt * 512 + tc4 * P
                for doc in range(4):
                    f_ps = ps_w.tile([P, 512], F32, tag="w", name=f"f{tt}_{tc4}_{doc}")
                    if len(lead) < 4:
                        for hh in range(3):
                            nc.tensor.matmul(f_ps[:],
                                             ot_sb[:, hh, tc4 * P:(tc4 + 1) * P],
                                             wo_sb[:, hh, doc * 512:(doc + 1) * 512],
                                             start=(hh == 0), stop=False)
                        lead.append((f_ps, tc4, doc, trow))
                        if len(lead) == 4:
                            finish_head(*last_norm)
                            for lf_ps, ltc4, ldoc, ltrow in lead:
                                nc.tensor.matmul(
                                    lf_ps[:],
                                    ot_sb[:, 3, ltc4 * P:(ltc4 + 1) * P],
                                    wo_sb[:, 3, ldoc * 512:(ldoc + 1) * 512],
                                    start=False, stop=True)
                                o_ev = oev_pool.tile([P, 512], F32, tag="oev",
                                                     name=f"oe{tt}_{ltc4}_{ldoc}")
                                if ldoc % 2 == 0:
                                    nc.vector.tensor_copy(o_ev[:], lf_ps[:])
                                else:
                                    nc.scalar.copy(o_ev[:], lf_ps[:])
                                nc.sync.dma_start(
                                    out=out[ltrow:ltrow + P,
                                            ldoc * 512:(ldoc + 1) * 512],
                                    in_=o_ev[:])
                        continue
                    for hh in range(4):
                        nc.tensor.matmul(f_ps[:],
                                         ot_sb[:, hh, tc4 * P:(tc4 + 1) * P],
                                         wo_sb[:, hh, doc * 512:(doc + 1) * 512],
                                         start=(hh == 0), stop=(hh == 3))
                    o_ev = oev_pool.tile([P, 512], F32, tag="oev",
                                         name=f"oe{tt}_{tc4}_{doc}")
                    if doc % 2 == 0:
                        nc.vector.tensor_copy(o_ev[:], f_ps[:])
                    else:
                        nc.scalar.copy(o_ev[:], f_ps[:])
                    nc.sync.dma_start(out=out[trow:trow + P, doc * 512:(doc + 1) * 512],
                                      in_=o_ev[:])
    nc.compile()
    return nc


def _host_tables():
    freqs = (1.0 / (np.float32(10000.0) **
                    (np.arange(0, HD, 2, dtype=np.float32) / np.float32(HD)))).astype(np.float32)
    t = np.arange(T, dtype=np.float32)
    ang = t[:, None] * freqs[None, :]
    cos = np.tile(np.cos(ang), (1, 2)).astype(np.float32)   # (T, HD)
    sin = np.tile(np.sin(ang), (1, 2)).astype(np.float32)
    cosT = np.ascontiguousarray(cos.T)                       # (HD, T)
    sinT = np.ascontiguousarray(sin.T)
    ssinT = sinT.copy()
    ssinT[:HD // 2] *= -1.0                                  # sign-folded sin
    # pure half-swap permutation; the rotate-half sign lives in ssinT
    rmat = np.zeros((P, P), dtype=np.float32)
    for j in range(HD // 2):
        rmat[j + HD // 2, j] = 1.0
    for j in range(HD // 2, HD):
        rmat[j - HD // 2, j] = 1.0
    tri = (np.arange(P)[:, None] <= np.arange(P)[None, :]).astype(np.float32)
    ident = np.eye(P, dtype=np.float32)
    return cosT, ssinT, rmat, tri, ident


def _make_in_maps(x, wq, wk, wv, wo):
    cosT, ssinT, rmat, tri, ident = _host_tables()
    x = np.asarray(x, dtype=np.float32)
    wq = np.asarray(wq, dtype=np.float32)
    wk = np.asarray(wk, dtype=np.float32)
    wv = np.asarray(wv, dtype=np.float32)
    wo = np.asarray(wo, dtype=np.float32)

    in_maps = []
    for c in range(8):
        b, h = divmod(c, 4)
        in_maps.append({
            "xT": np.ascontiguousarray(x[b].T).astype(NPDT),
            "wq": np.ascontiguousarray(wq[:, h * 512:(h + 1) * 512]).astype(NPDT),
            "wk": np.ascontiguousarray(wk[:, h * HD:(h + 1) * HD]).astype(NPDT),
            "wv": np.ascontiguousarray(wv[:, h * HD:(h + 1) * HD]).astype(NPDT),
            "wo": np.ascontiguousarray(wo[h * 512:(h + 1) * 512, :]).astype(NPDT),
            "cosT": cosT.astype(NPDT), "ssinT": ssinT.astype(NPDT),
            "rmat": rmat.astype(NPDT), "tri": tri.astype(NPDT),
            "ident": ident.astype(NPDT),
        })
    return in_maps


def kernel(x, wq, wk, wv, wo):
    if "nc" not in _cached:
        _cached["nc"] = _build()
    nc = _cached["nc"]
    in_maps = _make_in_maps(x, wq, wk, wv, wo)
    try:
        res = run_bass_kernel_spmd(nc, in_maps, core_ids=list(range(8)))
    except Exception:
        # transient NRT/device hiccups recover on a clean retry
        res = run_bass_kernel_spmd(nc, in_maps, core_ids=list(range(8)))
    outs = [res.results[c]["out"] for c in range(8)]
    full = np.stack([outs[0] + outs[1] + outs[2] + outs[3],
                     outs[4] + outs[5] + outs[6] + outs[7]], axis=0)
    return full.astype(np.float32)


# revision 31
# speedup vs baseline: 1.0001x; 1.0001x over previous
"""Causal GQA attention (B=2, T=2048, D=2048, QH=16, KVH=4, HD=128) on 8 TRN2 cores.

Sharding: DP-2 over batch x TP-4 over KV-head groups.
  core c -> batch c//4, kv head c%4, q heads 4*(c%4)..4*(c%4)+3.
Each core computes a partial (T, D) output (its heads' contribution through wo);
the host sums the 4 partials per batch (the all-reduce of the "wo along in dim"
sharding) and stacks the two batches.

Device dataflow (everything transposed; no on-device activation transposes):
  - host feeds xT = x[b].T                            (D, T)
  - qT/kT = W^T x computed directly in [hd, t] layout (wq chunks are lhsT)
  - RoPE via swap-permutation matmul (rot = R @ qT) + DVE mul/add with
    host cos / sign-folded-sin tables in [hd, t] layout
  - S^T[key, q] = (kT_blk)^T @ qT  per 128-key block  (one matmul, K=hd=128)
  - exp on ACT with fused 1/sqrt(hd) scale, PSUM -> SBUF f32r
  - causal: fully-masked column ranges of diagonal blocks are never computed;
    the 128x128 diagonal triangle is masked by a DVE multiply
  - O^T[hd, q] += V_blk^T @ expS^T   (V natural from 4 PE transposes per tile)
  - den_bcast[128, q] += ones128x128 @ expS^T  (accumulating matmul that sums
    over keys AND broadcasts the softmax denominator to all partitions)
  - normalize (deferred one head so the PE never stalls): den_bcast -> DVE
    evac -> chunked DVE reciprocal -> DVE multiply into O^T
  - out[t, d] = sum_h (OT_h)^T @ wo_h  accumulated over the 4 heads

All matmuls run in float16 (1 cycle/row on the PE; fp32 PSUM accumulation).
Measured on hardware: ~310 us HW exec, max rel err ~4.6e-4 vs the fp32
reference (scaled by output absmax).
"""
import numpy as np
from contextlib import ExitStack

import concourse.bacc as bacc
import concourse.tile as tile
import concourse.mybir as mybir
from concourse.bass_utils import run_bass_kernel_spmd

B, T, D = 2, 2048, 2048
QH, KVH = 16, 4
HD = D // QH            # 128
P = 128
NT = T // 512           # 4 t-tiles of 512
DC = D // P             # 16 contraction chunks
KB = T // P             # 16 key blocks
F32 = mybir.dt.float32
F32R = mybir.dt.float32r
CDT = mybir.dt.float16          # compute dtype on the PE (1 cycle/row)
NPDT = np.float16
AF = mybir.ActivationFunctionType
ALU = mybir.AluOpType
SCALE = float(1.0 / np.sqrt(HD))

_cached = {}


def _build():
    nc = bacc.Bacc("TRN2", target_bir_lowering=False, debug=False)
    xT = nc.dram_tensor("xT", [D, T], CDT, kind="ExternalInput")
    wq = nc.dram_tensor("wq", [D, 4 * HD], CDT, kind="ExternalInput")
    wk = nc.dram_tensor("wk", [D, HD], CDT, kind="ExternalInput")
    wv = nc.dram_tensor("wv", [D, HD], CDT, kind="ExternalInput")
    wo = nc.dram_tensor("wo", [4 * HD, D], CDT, kind="ExternalInput")
    cosT = nc.dram_tensor("cosT", [HD, T], CDT, kind="ExternalInput")
    ssinT = nc.dram_tensor("ssinT", [HD, T], CDT, kind="ExternalInput")
    rmat = nc.dram_tensor("rmat", [P, P], CDT, kind="ExternalInput")
    tri = nc.dram_tensor("tri", [P, P], CDT, kind="ExternalInput")
    ident = nc.dram_tensor("ident", [P, P], CDT, kind="ExternalInput")
    out = nc.dram_tensor("out", [T, D], F32, kind="ExternalOutput")

    with tile.TileContext(nc) as tc, ExitStack() as ctx:
        const = ctx.enter_context(tc.tile_pool(name="const", bufs=1))
        kvres = ctx.enter_context(tc.tile_pool(name="kvres", bufs=1))
        xc_pool = ctx.enter_context(tc.tile_pool(name="xc", bufs=6))
        qr_pool = ctx.enter_context(tc.tile_pool(name="qr", bufs=6))
        tmp_pool = ctx.enter_context(tc.tile_pool(name="tmp", bufs=3))
        e_pool = ctx.enter_context(tc.tile_pool(name="ep", bufs=8))
        ot_pool = ctx.enter_context(tc.tile_pool(name="ot", bufs=1))
        oev_pool = ctx.enter_context(tc.tile_pool(name="oev", bufs=4))
        bc_pool = ctx.enter_context(tc.tile_pool(name="bc", bufs=3))
        sm_pool = ctx.enter_context(tc.tile_pool(name="sm", bufs=2))

        ps_w = ctx.enter_context(tc.tile_pool(name="psw", bufs=5, space="PSUM"))
        ps_o = ctx.enter_context(tc.tile_pool(name="pso", bufs=2, space="PSUM"))
        ps_d = ctx.enter_context(tc.tile_pool(name="psd", bufs=1, space="PSUM"))

        # ---- resident constants (split + ordered for startup overlap) ----
        wq_sb = const.tile([P, DC, 4 * HD], CDT, tag="wq")
        wk_sb = const.tile([P, DC, HD], CDT, tag="wk")
        wv_sb = const.tile([P, DC, HD], CDT, tag="wv")
        wo_sb = const.tile([P, 4, D], CDT, tag="wo")
        cos_sb = const.tile([P, T], CDT, tag="cos")
        sin_sb = const.tile([P, T], CDT, tag="sin")
        rm_sb = const.tile([P, P], CDT, tag="rm")
        tri_sb = const.tile([P, P], CDT, tag="tri")
        id_sb = const.tile([P, P], CDT, tag="id")

        kT_all = kvres.tile([P, T], CDT, tag="kT")
        v_all = kvres.tile([P, KB, HD], CDT, tag="V")

        xT_v = xT.rearrange("(g dc p) t -> g p dc t", p=P, dc=4)   # 4-chunk groups
        wq_v = wq.rearrange("(g dc p) n -> g p dc n", p=P, dc=4)
        wk_v = wk.rearrange("(dc p) n -> p dc n", p=P)
        wv_v = wv.rearrange("(dc p) n -> p dc n", p=P)
        wo_v = wo.rearrange("(c p) n -> c p n", p=P)

        def load_x_tile(tt):
            grps = []
            for g in range(4):
                xg = xc_pool.tile([P, 4, 512], CDT, tag="xc", name=f"xc{tt}_{g}")
                nc.sync.dma_start(out=xg[:], in_=xT_v[g, :, :, tt * 512:(tt + 1) * 512])
                grps.append(xg)
            return [grps[dc // 4][:, dc % 4, :] for dc in range(DC)]

        # first t-tile's x groups interleaved with wq groups (startup critical);
        # the very first group is split in half so the first matmul starts early
        xgs0 = []
        for g in range(4):
            if g == 2:
                nc.sync.dma_start(out=cos_sb[:], in_=cosT[:])
                nc.sync.dma_start(out=sin_sb[:], in_=ssinT[:])
            xg = xc_pool.tile([P, 4, 512], CDT, tag="xc", name=f"xc0_{g}")
            if g == 0:
                nc.sync.dma_start(out=xg[:, 0:1, :], in_=xT_v[0, :, 0:1, 0:512])
                nc.sync.dma_start(out=wq_sb[:, 0:1, :], in_=wq_v[0, :, 0:1, :])
                nc.sync.dma_start(out=xg[:, 1:2, :], in_=xT_v[0, :, 1:2, 0:512])
                nc.sync.dma_start(out=wq_sb[:, 1:2, :], in_=wq_v[0, :, 1:2, :])
                nc.sync.dma_start(out=xg[:, 2:4, :], in_=xT_v[0, :, 2:4, 0:512])
                nc.sync.dma_start(out=wq_sb[:, 2:4, :], in_=wq_v[0, :, 2:4, :])
            else:
                nc.sync.dma_start(out=xg[:], in_=xT_v[g, :, :, 0:512])
                nc.sync.dma_start(out=wq_sb[:, g * 4:(g + 1) * 4, :], in_=wq_v[g])
            xgs0.append(xg)
        xcs0 = [xgs0[dc // 4][:, dc % 4, :] for dc in range(DC)]
        nc.sync.dma_start(out=wk_sb[:], in_=wk_v)
        nc.sync.dma_start(out=wv_sb[:], in_=wv_v)
        nc.sync.dma_start(out=rm_sb[:], in_=rmat[:])
        nc.sync.dma_start(out=tri_sb[:], in_=tri[:])
        nc.sync.dma_start(out=id_sb[:], in_=ident[:])
        ones_mat = const.tile([P, P], CDT, tag="ones")
        nc.vector.memset(ones_mat[:], 1.0)

        def rope(dst_ap, src_ps, tt, nm):
            """dst[hd, 512] = src*cos + (R@src)*ssin for t-tile tt. src is PSUM."""
            c_sl = cos_sb[:, tt * 512:(tt + 1) * 512]
            s_sl = sin_sb[:, tt * 512:(tt + 1) * 512]
            sb = tmp_pool.tile([P, 512], CDT, tag="evac", name=f"ev_{nm}")
            nc.scalar.copy(sb[:], src_ps[:])
            rot_ps = ps_w.tile([P, 512], F32, tag="w", name=f"rot_{nm}")
            nc.tensor.matmul(rot_ps[:], rm_sb[:], sb[:], start=True, stop=True)
            t1 = tmp_pool.tile([P, 512], F32, tag="t1", name=f"t1_{nm}")
            nc.vector.tensor_mul(t1[:], sb[:], c_sl)
            t2 = tmp_pool.tile([P, 512], F32, tag="t2", name=f"t2_{nm}")
            nc.vector.tensor_mul(t2[:], rot_ps[:], s_sl)
            with nc.allow_low_precision(reason="f32r rounding for PE"):
                nc.vector.tensor_add(dst_ap, t1[:], t2[:])

        def make_proj_thunks(tt, qT_roped):
            """A(tt) as a list of small emission thunks, paced into C(tt-1)."""
            xcs = xcs0 if tt == 0 else load_x_tile(tt)
            tsl = slice(tt * 512, (tt + 1) * 512)
            groups = [[("q", 0), ("q", 1)], [("q", 2), ("q", 3)],
                      [("k", 0), ("v", 0)]]
            thunks = []
            state = {}

            def finish_tgt(kind, idx, gi, tt=tt, tsl=tsl):
                ps = state[gi][(kind, idx)]
                if kind == "q":
                    rope(qT_roped[idx][:], ps, tt, f"q{tt}_{idx}")
                elif kind == "k":
                    rope(kT_all[:, tsl], ps, tt, f"k{tt}")
                else:
                    vt_sb = tmp_pool.tile([P, 512], CDT, tag="evac", name=f"vt{tt}")
                    nc.scalar.copy(vt_sb[:], ps[:])
                    tr_ps = ps_w.tile([P, 512], CDT, tag="w", name=f"vtr{tt}")
                    for i in range(4):
                        nc.tensor.transpose(tr_ps[:, i * P:(i + 1) * P],
                                            vt_sb[:, i * P:(i + 1) * P], id_sb[:])
                    for i in range(4):
                        with nc.allow_low_precision(reason="f32r store"):
                            nc.vector.tensor_copy(v_all[:, tt * 4 + i, :],
                                                  tr_ps[:, i * P:(i + 1) * P])

            for gi, grp in enumerate(groups):
                def alloc(grp=grp, gi=gi, tt=tt):
                    state[gi] = {
                        tgt: ps_w.tile([P, 512], F32, tag="w",
                                       name=f"proj{tt}_{tgt[0]}{tgt[1]}")
                        for tgt in grp}
                thunks.append(alloc)
                for dc in range(DC):
                    def mm(dc=dc, grp=grp, gi=gi):
                        for tgt in grp:
                            kind, idx = tgt
                            if kind == "q":
                                lhsT = wq_sb[:, dc, idx * HD:(idx + 1) * HD]
                            elif kind == "k":
                                lhsT = wk_sb[:, dc, :]
                            else:
                                lhsT = wv_sb[:, dc, :]
                            nc.tensor.matmul(state[gi][tgt][:], lhsT, xcs[dc],
                                             start=(dc == 0), stop=(dc == DC - 1))
                    thunks.append(mm)
                for tgt in grp:
                    thunks.append(lambda tgt=tgt, gi=gi: finish_tgt(*tgt, gi))
            return thunks

        qT_next = [qr_pool.tile([P, 512], CDT, tag="qr", name=f"qr0_{i}")
                   for i in range(4)]
        for th in make_proj_thunks(0, qT_next):   # A(0): nothing to hide behind
            th()
        thunks_next = []

        for tt in range(NT):
            tsl = slice(tt * 512, (tt + 1) * 512)
            qT_roped = qT_next
            if tt > 0:
                for th in make_proj_thunks(tt, qT_roped):
                    th()
            if tt + 1 < NT:
                qT_next = [qr_pool.tile([P, 512], CDT, tag="qr",
                                        name=f"qr{tt + 1}_{i}") for i in range(4)]

            # ---------- Phase B: attention, one-head-deferred normalization ----
            nkb = 4 * (tt + 1)
            ot_sb = ot_pool.tile([P, 4, 512], CDT, tag="ot", name=f"ot{tt}")
            pending_norm = []

            def finish_head(bc_ps, o_ps, hh, tt=tt, ot_sb=ot_sb):
                bc_sb = bc_pool.tile([P, 512], CDT, tag="bc", name=f"bs{tt}_{hh}")
                with nc.allow_low_precision(reason="bc evac"):
                    nc.vector.tensor_copy(bc_sb[:], bc_ps[:])
                rb_sb = bc_pool.tile([P, 512], CDT, tag="rb", name=f"rb{tt}_{hh}")
                # chunked so phase C's first consumers unblock after 1/4 of the
                # reciprocal instead of the whole 3.3us op
                for ch in range(4):
                    cs = slice(ch * P, (ch + 1) * P)
                    with nc.allow_low_precision(reason="recip"):
                        nc.vector.reciprocal(rb_sb[:, cs], bc_sb[:, cs])
                    with nc.allow_low_precision(reason="norm"):
                        nc.vector.tensor_mul(ot_sb[:, hh, cs], o_ps[:, cs],
                                             rb_sb[:, cs])

            for hh in range(4):
                o_ps = ps_o.tile([P, 512], F32, tag="o", name=f"o{tt}_{hh}")
                den_ps = ps_d.tile([P, 512], F32, tag="den", name=f"d{tt}_{hh}")
                prev = None   # (kb, lo, e_sb)
                for kb in range(nkb):
                    di = kb - 4 * tt          # >=0 on diagonal blocks
                    lo = di * P if di > 0 else 0
                    s_ps = ps_w.tile([P, 512], F32, tag="w", name=f"s{tt}_{hh}_{kb}")
                    nc.tensor.matmul(s_ps[:, lo:512],
                                     kT_all[:, kb * P:(kb + 1) * P],
                                     qT_roped[hh][:, lo:512], start=True, stop=True)
                    e_sb = e_pool.tile([P, 512], CDT, tag="e", name=f"e{tt}_{hh}_{kb}")
                    nc.scalar.activation(e_sb[:, lo:512], s_ps[:, lo:512], AF.Exp,
                                         scale=SCALE)
                    if di >= 0:
                        with nc.allow_low_precision(reason="mask mult"):
                            nc.vector.tensor_mul(e_sb[:, di * P:(di + 1) * P],
                                                 e_sb[:, di * P:(di + 1) * P],
                                                 tri_sb[:])
                    if prev is not None:
                        pkb, plo, pe = prev
                        nc.tensor.matmul(o_ps[:, plo:512], v_all[:, pkb, :],
                                         pe[:, plo:512],
                                         start=(pkb == 0), stop=False)
                        nc.tensor.matmul(den_ps[:, plo:512], ones_mat,
                                         pe[:, plo:512],
                                         start=(pkb == 0), stop=False)
                    prev = (kb, lo, e_sb)
                pkb, plo, pe = prev
                nc.tensor.matmul(o_ps[:, plo:512], v_all[:, pkb, :], pe[:, plo:512],
                                 start=(pkb == 0), stop=True)
                nc.tensor.matmul(den_ps[:, plo:512], ones_mat, pe[:, plo:512],
                                 start=(pkb == 0), stop=True)
                # den_ps already holds the broadcast denominators; defer
                # evac+recip+norm one head so the PE never waits
                if pending_norm:
                    finish_head(*pending_norm.pop())
                pending_norm.append((den_ps, o_ps, hh))
            last_norm = pending_norm.pop()

            if tt == 0:
                # wo arrives late on purpose: keeps startup DMA bandwidth for
                # the tensors the first projections need
                for c in range(4):
                    nc.sync.dma_start(out=wo_sb[:, c, :], in_=wo_v[c])

            # ---------- Phase C: output projection ----------
            # First two f-groups: emit heads 0-2, then the deferred head-3
            # normalization (its reciprocal overlaps these matmuls), then the
            # head-3 contributions.
            lead = []
            for tc4 in range(4):
                trow = tt * 512 + tc4 * P
                for doc in range(4):
                    f_ps = ps_w.tile([P, 512], F32, tag="w", name=f"f{tt}_{tc4}_{doc}")
                    if len(lead) < 4:
                        for hh in range(3):
                            nc.tensor.matmul(f_ps[:],
                                             ot_sb[:, hh, tc4 * P:(tc4 + 1) * P],
                                             wo_sb[:, hh, doc * 512:(doc + 1) * 512],
                                             start=(hh == 0), stop=False)
                        lead.append((f_ps, tc4, doc, trow))
                        if len(lead) == 4:
                            finish_head(*last_norm)
                            for lf_ps, ltc4, ldoc, ltrow in lead:
                                nc.tensor.matmul(
                                    lf_ps[:],
                                    ot_sb[:, 3, ltc4 * P:(ltc4 + 1) * P],
                                    wo_sb[:, 3, ldoc * 512:(ldoc + 1) * 512],
                                    start=False, stop=True)
                                o_ev = oev_pool.tile([P, 512], F32, tag="oev",
                                                     name=f"oe{tt}_{ltc4}_{ldoc}")
                                if ldoc % 2 == 0:
                                    nc.vector.tensor_copy(o_ev[:], lf_ps[:])
                                else:
                                    nc.scalar.copy(o_ev[:], lf_ps[:])
                                nc.sync.dma_start(
                                    out=out[ltrow:ltrow + P,
                                            ldoc * 512:(ldoc + 1) * 512],
                                    in_=o_ev[:])
                        continue
                    for hh in range(4):
                        nc.tensor.matmul(f_ps[:],
                                         ot_sb[:, hh, tc4 * P:(tc4 + 1) * P],
                                         wo_sb[:, hh, doc * 512:(doc + 1) * 512],
                                         start=(hh == 0), stop=(hh == 3))
                    o_ev = oev_pool.tile([P, 512], F32, tag="oev",
                                         name=f"oe{tt}_{tc4}_{doc}")
                    if doc % 2 == 0:
                        nc.vector.tensor_copy(o_ev[:], f_ps[:])
                    else:
                        nc.scalar.copy(o_ev[:], f_ps[:])
                    nc.sync.dma_start(out=out[trow:trow + P, doc * 512:(doc + 1) * 512],
                                      in_=o_ev[:])
    nc.compile()
    return nc


def _host_tables():
    freqs = (1.0 / (np.float32(10000.0) **
                    (np.arange(0, HD, 2, dtype=np.float32) / np.float32(HD)))).astype(np.float32)
    t = np.arange(T, dtype=np.float32)
    ang = t[:, None] * freqs[None, :]
    cos = np.tile(np.cos(ang), (1, 2)).astype(np.float32)   # (T, HD)
    sin = np.tile(np.sin(ang), (1, 2)).astype(np.float32)
    cosT = np.ascontiguousarray(cos.T)                       # (HD, T)
    sinT = np.ascontiguousarray(sin.T)
    ssinT = sinT.copy()
    ssinT[:HD // 2] *= -1.0                                  # sign-folded sin
    # pure half-swap permutation; the rotate-half sign lives in ssinT
    rmat = np.zeros((P, P), dtype=np.float32)
    for j in range(HD // 2):
        rmat[j + HD // 2, j] = 1.0
    for j in range(HD // 2, HD):
        rmat[j - HD // 2, j] = 1.0
    tri = (np.arange(P)[:, None] <= np.arange(P)[None, :]).astype(np.float32)
    ident = np.eye(P, dtype=np.float32)
    return cosT, ssinT, rmat, tri, ident


def _make_in_maps(x, wq, wk, wv, wo):
    cosT, ssinT, rmat, tri, ident = _host_tables()
    x = np.asarray(x, dtype=np.float32)
    wq = np.asarray(wq, dtype=np.float32)
    wk = np.asarray(wk, dtype=np.float32)
    wv = np.asarray(wv, dtype=np.float32)
    wo = np.asarray(wo, dtype=np.float32)

    in_maps = []
    for c in range(8):
        b, h = divmod(c, 4)
        in_maps.append({
            "xT": np.ascontiguousarray(x[b].T).astype(NPDT),
            "wq": np.ascontiguousarray(wq[:, h * 512:(h + 1) * 512]).astype(NPDT),
            "wk": np.ascontiguousarray(wk[:, h * HD:(h + 1) * HD]).astype(NPDT),
            "wv": np.ascontiguousarray(wv[:, h * HD:(h + 1) * HD]).astype(NPDT),
            "wo": np.ascontiguousarray(wo[h * 512:(h + 1) * 512, :]).astype(NPDT),
            "cosT": cosT.astype(NPDT), "ssinT": ssinT.astype(NPDT),
            "rmat": rmat.astype(NPDT), "tri": tri.astype(NPDT),
            "ident": ident.astype(NPDT),
        })
    return in_maps


def kernel(x, wq, wk, wv, wo):
    if "nc" not in _cached:
        _cached["nc"] = _build()
    nc = _cached["nc"]
    in_maps = _make_in_maps(x, wq, wk, wv, wo)
    try:
        res = run_bass_kernel_spmd(nc, in_maps, core_ids=list(range(8)))
    except Exception:
        # transient NRT/device hiccups recover on a clean retry
        res = run_bass_kernel_spmd(nc, in_maps, core_ids=list(range(8)))
    outs = [res.results[c]["out"] for c in range(8)]
    full = np.stack([outs[0] + outs[1] + outs[2] + outs[3],
                     outs[4] + outs[5] + outs[6] + outs[7]], axis=0)
    return full.astype(np.float32)


# revision 32
# speedup vs baseline: 1.0175x; 1.0174x over previous
"""Causal GQA attention (B=2, T=2048, D=2048, QH=16, KVH=4, HD=128) on 8 TRN2 cores.

Sharding: DP-2 over batch x TP-4 over KV-head groups.
  core c -> batch c//4, kv head c%4, q heads 4*(c%4)..4*(c%4)+3.
Each core computes a partial (T, D) output (its heads' contribution through wo);
the host sums the 4 partials per batch (the all-reduce of the "wo along in dim"
sharding) and stacks the two batches.

Device dataflow (everything transposed; no on-device activation transposes):
  - host feeds xT = x[b].T                            (D, T)
  - qT/kT = W^T x computed directly in [hd, t] layout (wq chunks are lhsT)
  - RoPE via swap-permutation matmul (rot = R @ qT) + DVE mul/add with
    host cos / sign-folded-sin tables in [hd, t] layout
  - S^T[key, q] = (kT_blk)^T @ qT  per 128-key block  (one matmul, K=hd=128)
  - exp on ACT with fused 1/sqrt(hd) scale, PSUM -> SBUF f32r
  - causal: fully-masked column ranges of diagonal blocks are never computed;
    the 128x128 diagonal triangle is masked by a DVE multiply
  - O^T[hd, q] += V_blk^T @ expS^T   (V natural from 4 PE transposes per tile)
  - den_bcast[128, q] += ones128x128 @ expS^T  (accumulating matmul that sums
    over keys AND broadcasts the softmax denominator to all partitions)
  - normalize (deferred one head so the PE never stalls): den_bcast -> DVE
    evac -> chunked DVE reciprocal -> DVE multiply into O^T
  - out[t, d] = sum_h (OT_h)^T @ wo_h  accumulated over the 4 heads

All matmuls run in float16 (1 cycle/row on the PE; fp32 PSUM accumulation).
Measured on hardware: ~310 us HW exec, max rel err ~4.6e-4.
"""
import numpy as np
from contextlib import ExitStack

import concourse.bacc as bacc
import concourse.tile as tile
import concourse.mybir as mybir
from concourse.bass_utils import run_bass_kernel_spmd

B, T, D = 2, 2048, 2048
QH, KVH = 16, 4
HD = D // QH            # 128
P = 128
NT = T // 512           # 4 t-tiles of 512
DC = D // P             # 16 contraction chunks
KB = T // P             # 16 key blocks
F32 = mybir.dt.float32
F32R = mybir.dt.float32r
CDT = mybir.dt.float16          # compute dtype on the PE (1 cycle/row)
NPDT = np.float16
AF = mybir.ActivationFunctionType
ALU = mybir.AluOpType
SCALE = float(1.0 / np.sqrt(HD))

_cached = {}


def _build():
    nc = bacc.Bacc("TRN2", target_bir_lowering=False, debug=False)
    xT = nc.dram_tensor("xT", [D, T], CDT, kind="ExternalInput")
    wq = nc.dram_tensor("wq", [D, 4 * HD], CDT, kind="ExternalInput")
    wk = nc.dram_tensor("wk", [D, HD], CDT, kind="ExternalInput")
    wv = nc.dram_tensor("wv", [D, HD], CDT, kind="ExternalInput")
    wo = nc.dram_tensor("wo", [4 * HD, D], CDT, kind="ExternalInput")
    cosT = nc.dram_tensor("cosT", [HD, T], CDT, kind="ExternalInput")
    ssinT = nc.dram_tensor("ssinT", [HD, T], CDT, kind="ExternalInput")
    rmat = nc.dram_tensor("rmat", [P, P], CDT, kind="ExternalInput")
    tri = nc.dram_tensor("tri", [P, P], CDT, kind="ExternalInput")
    ident = nc.dram_tensor("ident", [P, P], CDT, kind="ExternalInput")
    out = nc.dram_tensor("out", [T, D], F32, kind="ExternalOutput")

    with tile.TileContext(nc) as tc, ExitStack() as ctx:
        const = ctx.enter_context(tc.tile_pool(name="const", bufs=1))
        kvres = ctx.enter_context(tc.tile_pool(name="kvres", bufs=1))
        xc_pool = ctx.enter_context(tc.tile_pool(name="xc", bufs=6))
        qr_pool = ctx.enter_context(tc.tile_pool(name="qr", bufs=6))
        tmp_pool = ctx.enter_context(tc.tile_pool(name="tmp", bufs=3))
        e_pool = ctx.enter_context(tc.tile_pool(name="ep", bufs=8))
        ot_pool = ctx.enter_context(tc.tile_pool(name="ot", bufs=1))
        oev_pool = ctx.enter_context(tc.tile_pool(name="oev", bufs=4))
        bc_pool = ctx.enter_context(tc.tile_pool(name="bc", bufs=3))
        sm_pool = ctx.enter_context(tc.tile_pool(name="sm", bufs=2))

        ps_w = ctx.enter_context(tc.tile_pool(name="psw", bufs=5, space="PSUM"))
        ps_o = ctx.enter_context(tc.tile_pool(name="pso", bufs=2, space="PSUM"))
        ps_d = ctx.enter_context(tc.tile_pool(name="psd", bufs=1, space="PSUM"))

        # ---- resident constants (split + ordered for startup overlap) ----
        wq_sb = const.tile([P, DC, 4 * HD], CDT, tag="wq")
        wk_sb = const.tile([P, DC, HD], CDT, tag="wk")
        wv_sb = const.tile([P, DC, HD], CDT, tag="wv")
        wo_sb = const.tile([P, 4, D], CDT, tag="wo")
        cos_sb = const.tile([P, T], CDT, tag="cos")
        sin_sb = const.tile([P, T], CDT, tag="sin")
        rm_sb = const.tile([P, P], CDT, tag="rm")
        tri_sb = const.tile([P, P], CDT, tag="tri")
        id_sb = const.tile([P, P], CDT, tag="id")

        kT_all = kvres.tile([P, T], CDT, tag="kT")
        v_all = kvres.tile([P, KB, HD], CDT, tag="V")

        xT_v = xT.rearrange("(g dc p) t -> g p dc t", p=P, dc=4)   # 4-chunk groups
        wq_v = wq.rearrange("(g dc p) n -> g p dc n", p=P, dc=4)
        wk_v = wk.rearrange("(dc p) n -> p dc n", p=P)
        wv_v = wv.rearrange("(dc p) n -> p dc n", p=P)
        wo_v = wo.rearrange("(c p) n -> c p n", p=P)

        def load_x_tile(tt):
            grps = []
            for g in range(4):
                xg = xc_pool.tile([P, 4, 512], CDT, tag="xc", name=f"xc{tt}_{g}")
                nc.sync.dma_start(out=xg[:], in_=xT_v[g, :, :, tt * 512:(tt + 1) * 512])
                grps.append(xg)
            return [grps[dc // 4][:, dc % 4, :] for dc in range(DC)]

        # first t-tile's x groups interleaved with wq groups (startup critical);
        # the very first group is split in half so the first matmul starts early
        xgs0 = []
        for g in range(4):
            if g == 2:
                nc.sync.dma_start(out=cos_sb[:], in_=cosT[:])
                nc.sync.dma_start(out=sin_sb[:], in_=ssinT[:])
            xg = xc_pool.tile([P, 4, 512], CDT, tag="xc", name=f"xc0_{g}")
            if g == 0:
                nc.sync.dma_start(out=xg[:, 0:2, :], in_=xT_v[0, :, 0:2, 0:512])
                nc.sync.dma_start(out=wq_sb[:, 0:2, :], in_=wq_v[0, :, 0:2, :])
                nc.sync.dma_start(out=xg[:, 2:4, :], in_=xT_v[0, :, 2:4, 0:512])
                nc.sync.dma_start(out=wq_sb[:, 2:4, :], in_=wq_v[0, :, 2:4, :])
            else:
                nc.sync.dma_start(out=xg[:], in_=xT_v[g, :, :, 0:512])
                nc.sync.dma_start(out=wq_sb[:, g * 4:(g + 1) * 4, :], in_=wq_v[g])
            xgs0.append(xg)
        xcs0 = [xgs0[dc // 4][:, dc % 4, :] for dc in range(DC)]
        nc.sync.dma_start(out=wk_sb[:], in_=wk_v)
        nc.sync.dma_start(out=wv_sb[:], in_=wv_v)
        nc.sync.dma_start(out=rm_sb[:], in_=rmat[:])
        nc.sync.dma_start(out=tri_sb[:], in_=tri[:])
        nc.sync.dma_start(out=id_sb[:], in_=ident[:])
        ones_mat = const.tile([P, P], CDT, tag="ones")
        nc.vector.memset(ones_mat[:], 1.0)

        def rope(dst_ap, src_ps, tt, nm):
            """dst[hd, 512] = src*cos + (R@src)*ssin for t-tile tt. src is PSUM."""
            c_sl = cos_sb[:, tt * 512:(tt + 1) * 512]
            s_sl = sin_sb[:, tt * 512:(tt + 1) * 512]
            sb = tmp_pool.tile([P, 512], CDT, tag="evac", name=f"ev_{nm}")
            nc.scalar.copy(sb[:], src_ps[:])
            rot_ps = ps_w.tile([P, 512], F32, tag="w", name=f"rot_{nm}")
            nc.tensor.matmul(rot_ps[:], rm_sb[:], sb[:], start=True, stop=True)
            t1 = tmp_pool.tile([P, 512], F32, tag="t1", name=f"t1_{nm}")
            nc.vector.tensor_mul(t1[:], sb[:], c_sl)
            t2 = tmp_pool.tile([P, 512], F32, tag="t2", name=f"t2_{nm}")
            nc.vector.tensor_mul(t2[:], rot_ps[:], s_sl)
            with nc.allow_low_precision(reason="f32r rounding for PE"):
                nc.vector.tensor_add(dst_ap, t1[:], t2[:])

        def make_proj_thunks(tt, qT_roped):
            """A(tt) as a list of small emission thunks, paced into C(tt-1)."""
            xcs = xcs0 if tt == 0 else load_x_tile(tt)
            tsl = slice(tt * 512, (tt + 1) * 512)
            groups = [[("q", 0), ("q", 1)], [("q", 2), ("q", 3)],
                      [("k", 0), ("v", 0)]]
            thunks = []
            state = {}

            def finish_tgt(kind, idx, gi, tt=tt, tsl=tsl):
                ps = state[gi][(kind, idx)]
                if kind == "q":
                    rope(qT_roped[idx][:], ps, tt, f"q{tt}_{idx}")
                elif kind == "k":
                    rope(kT_all[:, tsl], ps, tt, f"k{tt}")
                else:
                    vt_sb = tmp_pool.tile([P, 512], CDT, tag="evac", name=f"vt{tt}")
                    nc.scalar.copy(vt_sb[:], ps[:])
                    tr_ps = ps_w.tile([P, 512], CDT, tag="w", name=f"vtr{tt}")
                    for i in range(4):
                        nc.tensor.transpose(tr_ps[:, i * P:(i + 1) * P],
                                            vt_sb[:, i * P:(i + 1) * P], id_sb[:])
                    for i in range(4):
                        with nc.allow_low_precision(reason="f32r store"):
                            nc.vector.tensor_copy(v_all[:, tt * 4 + i, :],
                                                  tr_ps[:, i * P:(i + 1) * P])

            for gi, grp in enumerate(groups):
                def alloc(grp=grp, gi=gi, tt=tt):
                    state[gi] = {
                        tgt: ps_w.tile([P, 512], F32, tag="w",
                                       name=f"proj{tt}_{tgt[0]}{tgt[1]}")
                        for tgt in grp}
                thunks.append(alloc)
                for dc in range(DC):
                    def mm(dc=dc, grp=grp, gi=gi):
                        for tgt in grp:
                            kind, idx = tgt
                            if kind == "q":
                                lhsT = wq_sb[:, dc, idx * HD:(idx + 1) * HD]
                            elif kind == "k":
                                lhsT = wk_sb[:, dc, :]
                            else:
                                lhsT = wv_sb[:, dc, :]
                            nc.tensor.matmul(state[gi][tgt][:], lhsT, xcs[dc],
                                             start=(dc == 0), stop=(dc == DC - 1))
                    thunks.append(mm)
                for tgt in grp:
                    thunks.append(lambda tgt=tgt, gi=gi: finish_tgt(*tgt, gi))
            return thunks

        qT_next = [qr_pool.tile([P, 512], CDT, tag="qr", name=f"qr0_{i}")
                   for i in range(4)]
        for th in make_proj_thunks(0, qT_next):   # A(0): nothing to hide behind
            th()
        thunks_next = []

        for tt in range(NT):
            tsl = slice(tt * 512, (tt + 1) * 512)
            qT_roped = qT_next
            if tt > 0:
                for th in make_proj_thunks(tt, qT_roped):
                    th()
            if tt + 1 < NT:
                qT_next = [qr_pool.tile([P, 512], CDT, tag="qr",
                                        name=f"qr{tt + 1}_{i}") for i in range(4)]

            # ---------- Phase B: attention, one-head-deferred normalization ----
            nkb = 4 * (tt + 1)
            ot_sb = ot_pool.tile([P, 4, 512], CDT, tag="ot", name=f"ot{tt}")
            pending_norm = []

            def finish_head(bc_ps, o_ps, hh, tt=tt, ot_sb=ot_sb):
                bc_sb = bc_pool.tile([P, 512], CDT, tag="bc", name=f"bs{tt}_{hh}")
                with nc.allow_low_precision(reason="bc evac"):
                    nc.vector.tensor_copy(bc_sb[:], bc_ps[:])
                rb_sb = bc_pool.tile([P, 512], CDT, tag="rb", name=f"rb{tt}_{hh}")
                # chunked so phase C's first consumers unblock after 1/4 of the
                # reciprocal instead of the whole 3.3us op
                for ch in range(4):
                    cs = slice(ch * P, (ch + 1) * P)
                    with nc.allow_low_precision(reason="recip"):
                        nc.vector.reciprocal(rb_sb[:, cs], bc_sb[:, cs])
                    with nc.allow_low_precision(reason="norm"):
                        nc.vector.tensor_mul(ot_sb[:, hh, cs], o_ps[:, cs],
                                             rb_sb[:, cs])

            for hh in range(4):
                o_ps = ps_o.tile([P, 512], F32, tag="o", name=f"o{tt}_{hh}")
                den_ps = ps_d.tile([P, 512], F32, tag="den", name=f"d{tt}_{hh}")
                prev = None   # (kb, lo, e_sb)
                for kb in range(nkb):
                    di = kb - 4 * tt          # >=0 on diagonal blocks
                    lo = di * P if di > 0 else 0
                    s_ps = ps_w.tile([P, 512], F32, tag="w", name=f"s{tt}_{hh}_{kb}")
                    nc.tensor.matmul(s_ps[:, lo:512],
                                     kT_all[:, kb * P:(kb + 1) * P],
                                     qT_roped[hh][:, lo:512], start=True, stop=True)
                    e_sb = e_pool.tile([P, 512], CDT, tag="e", name=f"e{tt}_{hh}_{kb}")
                    nc.scalar.activation(e_sb[:, lo:512], s_ps[:, lo:512], AF.Exp,
                                         scale=SCALE)
                    if di >= 0:
                        with nc.allow_low_precision(reason="mask mult"):
                            nc.vector.tensor_mul(e_sb[:, di * P:(di + 1) * P],
                                                 e_sb[:, di * P:(di + 1) * P],
                                                 tri_sb[:])
                    if prev is not None:
                        pkb, plo, pe = prev
                        nc.tensor.matmul(o_ps[:, plo:512], v_all[:, pkb, :],
                                         pe[:, plo:512],
                                         start=(pkb == 0), stop=False)
                        nc.tensor.matmul(den_ps[:, plo:512], ones_mat,
                                         pe[:, plo:512],
                                         start=(pkb == 0), stop=False)
                    prev = (kb, lo, e_sb)
                pkb, plo, pe = prev
                nc.tensor.matmul(o_ps[:, plo:512], v_all[:, pkb, :], pe[:, plo:512],
                                 start=(pkb == 0), stop=True)
                nc.tensor.matmul(den_ps[:, plo:512], ones_mat, pe[:, plo:512],
                                 start=(pkb == 0), stop=True)
                # den_ps already holds the broadcast denominators; defer
                # evac+recip+norm one head so the PE never waits
                if pending_norm:
                    finish_head(*pending_norm.pop())
                pending_norm.append((den_ps, o_ps, hh))
            last_norm = pending_norm.pop()

            if tt == 0:
                # wo arrives late on purpose: keeps startup DMA bandwidth for
                # the tensors the first projections need
                for c in range(4):
                    nc.sync.dma_start(out=wo_sb[:, c, :], in_=wo_v[c])

            # ---------- Phase C: output projection ----------
            # First two f-groups: emit heads 0-2, then the deferred head-3
            # normalization (its reciprocal overlaps these matmuls), then the
            # head-3 contributions.
            lead = []
            for tc4 in range(4):
                trow = tt * 512 + tc4 * P
                for doc in range(4):
                    f_ps = ps_w.tile([P, 512], F32, tag="w", name=f"f{tt}_{tc4}_{doc}")
                    if len(lead) < 4:
                        for hh in range(3):
                            nc.tensor.matmul(f_ps[:],
                                             ot_sb[:, hh, tc4 * P:(tc4 + 1) * P],
                                             wo_sb[:, hh, doc * 512:(doc + 1) * 512],
                                             start=(hh == 0), stop=False)
                        lead.append((f_ps, tc4, doc, trow))
                        if len(lead) == 4:
                            finish_head(*last_norm)
                            for lf_ps, ltc4, ldoc, ltrow in lead:
                                nc.tensor.matmul(
                                    lf_ps[:],
                                    ot_sb[:, 3, ltc4 * P:(ltc4 + 1) * P],
                                    wo_sb[:, 3, ldoc * 512:(ldoc + 1) * 512],
                                    start=False, stop=True)
                                o_ev = oev_pool.tile([P, 512], F32, tag="oev",
                                                     name=f"oe{tt}_{ltc4}_{ldoc}")
                                if ldoc % 2 == 0:
                                    nc.vector.tensor_copy(o_ev[:], lf_ps[:])
                                else:
                                    nc.scalar.copy(o_ev[:], lf_ps[:])
                                nc.sync.dma_start(
                                    out=out[ltrow:ltrow + P,
                                            ldoc * 512:(ldoc + 1) * 512],
                                    in_=o_ev[:])
                        continue
                    for hh in range(4):
                        nc.tensor.matmul(f_ps[:],
                                         ot_sb[:, hh, tc4 * P:(tc4 + 1) * P],
                                         wo_sb[:, hh, doc * 512:(doc + 1) * 512],
                                         start=(hh == 0), stop=(hh == 3))
                    o_ev = oev_pool.tile([P, 512], F32, tag="oev",
                                         name=f"oe{tt}_{tc4}_{doc}")
                    if doc % 2 == 0:
                        nc.vector.tensor_copy(o_ev[:], f_ps[:])
                    else:
                        nc.scalar.copy(o_ev[:], f_ps[:])
                    nc.sync.dma_start(out=out[trow:trow + P, doc * 512:(doc + 1) * 512],
                                      in_=o_ev[:])
    nc.compile()
    return nc


def _host_tables():
    freqs = (1.0 / (np.float32(10000.0) **
                    (np.arange(0, HD, 2, dtype=np.float32) / np.float32(HD)))).astype(np.float32)
    t = np.arange(T, dtype=np.float32)
    ang = t[:, None] * freqs[None, :]
    cos = np.tile(np.cos(ang), (1, 2)).astype(np.float32)   # (T, HD)
    sin = np.tile(np.sin(ang), (1, 2)).astype(np.float32)
    cosT = np.ascontiguousarray(cos.T)                       # (HD, T)
    sinT = np.ascontiguousarray(sin.T)
    ssinT = sinT.copy()
    ssinT[:HD // 2] *= -1.0                                  # sign-folded sin
    # pure half-swap permutation; the rotate-half sign lives in ssinT
    rmat = np.zeros((P, P), dtype=np.float32)
    for j in range(HD // 2):
        rmat[j + HD // 2, j] = 1.0
    for j in range(HD // 2, HD):
        rmat[j - HD // 2, j] = 1.0
    tri = (np.arange(P)[:, None] <= np.arange(P)[None, :]).astype(np.float32)
    ident = np.eye(P, dtype=np.float32)
    return cosT, ssinT, rmat, tri, ident


def _make_in_maps(x, wq, wk, wv, wo):
    cosT, ssinT, rmat, tri, ident = _host_tables()
    x = np.asarray(x, dtype=np.float32)
    wq = np.asarray(wq, dtype=np.float32)
    wk = np.asarray(wk, dtype=np.float32)
    wv = np.asarray(wv, dtype=np.float32)
    wo = np.asarray(wo, dtype=np.float32)

    in_maps = []
    for c in range(8):
        b, h = divmod(c, 4)
        in_maps.append({
            "xT": np.ascontiguousarray(x[b].T).astype(NPDT),
            "wq": np.ascontiguousarray(wq[:, h * 512:(h + 1) * 512]).astype(NPDT),
            "wk": np.ascontiguousarray(wk[:, h * HD:(h + 1) * HD]).astype(NPDT),
            "wv": np.ascontiguousarray(wv[:, h * HD:(h + 1) * HD]).astype(NPDT),
            "wo": np.ascontiguousarray(wo[h * 512:(h + 1) * 512, :]).astype(NPDT),
            "cosT": cosT.astype(NPDT), "ssinT": ssinT.astype(NPDT),
            "rmat": rmat.astype(NPDT), "tri": tri.astype(NPDT),
            "ident": ident.astype(NPDT),
        })
    return in_maps


def kernel(x, wq, wk, wv, wo):
    if "nc" not in _cached:
        _cached["nc"] = _build()
    nc = _cached["nc"]
    in_maps = _make_in_maps(x, wq, wk, wv, wo)
    try:
        res = run_bass_kernel_spmd(nc, in_maps, core_ids=list(range(8)))
    except Exception:
        # transient NRT/device hiccups recover on a clean retry
        res = run_bass_kernel_spmd(nc, in_maps, core_ids=list(range(8)))
    outs = [res.results[c]["out"] for c in range(8)]
    full = np.stack([outs[0] + outs[1] + outs[2] + outs[3],
                     outs[4] + outs[5] + outs[6] + outs[7]], axis=0)
    return full.astype(np.float32)
